# revision 5
# baseline (speedup 1.0000x reference)
"""Trainium2 Bass kernel for nn_DMS_STAttention_48722109006179.

Mathematical note (load-bearing): in the reference, `_attention_layer`
ends with softmax over axis=-1, which is the head dimension of size
H=1.  Softmax over a single-element axis is identically 1.0, so the
entire attention computation (linears, edge gather/scatter, LeakyReLU)
collapses and the outputs are exactly

    sa[b, t, i, j, 0] = 1.0 + sa_bias[t, i, j]
    ta[b, j, s, t, 0] = 1.0 + ta_bias[j, s, t]

independent of `src` and all weights (verified bit-exact against the
jax reference on device).  The kernel is therefore a pure memory-regime
problem: materialize ~58 MB of broadcast output.  Sharding: pure data
parallel over batch B=2048 across 8 cores (256 batch rows per core).

Per-core device program (raw bass — the toolchain here encodes at most
ONE semaphore wait per instruction, which rules out TileContext's
auto-drain; explicit standalone wait_ge instructions are used instead):
  1. HWDGE DMA of [ones(128) | sa_bias | ta_bias] row into SBUF.
  2. TensorEngine K=1 ones-matmuls broadcast the row across 128
     partitions into PSUM (bank-sized chunks).
  3. VectorEngine copies PSUM->SBUF with the +1.0 fused.
  4. HWDGE DMAs write the replicated [128, N] tiles to the DRAM
     outputs twice each (256 batch rows per core).
"""

import numpy as np

import concourse.bass as bass
from concourse import mybir
from concourse.bass_utils import run_bass_kernel_spmd

N_CORES = 8
B = 2048
T = 10
J = 22
SA = T * J * J  # 4840
TA = J * T * T  # 2200
BPC = B // N_CORES  # 256 batch rows per core

_SA_CHUNK = 484  # 10 sa chunks
_TA_CHUNK = 440  # 5 ta chunks
_N_SA = SA // _SA_CHUNK
_N_TA = TA // _TA_CHUNK
_N_CHUNKS = _N_SA + _N_TA

# test.py hooks (ignored by the grading harness)
TRACE = False
LAST_EXEC_NS = None

_NC_CACHE = {}


def _build_nc():
    nc = bass.Bass()
    f32 = mybir.dt.float32

    # bias_cat layout: [ones(128) | sa_bias(4840) | ta_bias(2200)]
    bias_cat = nc.dram_tensor(
        "bias_cat", [1, 128 + SA + TA], f32, kind="ExternalInput"
    )
    out_sa = nc.dram_tensor("out_sa", [BPC, SA], f32, kind="ExternalOutput")
    out_ta = nc.dram_tensor("out_ta", [BPC, TA], f32, kind="ExternalOutput")

    # chunk table: (sbuf col base in t_all, src col base in brow, width)
    chunks = [(c, 128 + c, _SA_CHUNK) for c in range(0, SA, _SA_CHUNK)] + [
        (SA + c, 128 + SA + c, _TA_CHUNK) for c in range(0, TA, _TA_CHUNK)
    ]

    with (
        nc.semaphore("s_in") as s_in,
        nc.semaphore("s_mm") as s_mm,
        nc.semaphore("s_cp") as s_cp,
        nc.semaphore("s_out") as s_out,
        nc.sbuf_tensor("brow", [1, 128 + SA + TA], f32) as brow,
        nc.sbuf_tensor("t_all", [128, SA + TA], f32) as t_all,
        nc.psum_tensor("acc", [128, 4096], f32) as acc,
    ):
        with nc.Block() as block:

            @block.sync
            def _(sync):
                sync.dma_start(out=brow[:], in_=bias_cat[:]).then_inc(s_in, 16)
                # sa ready after the first _N_SA copies
                sync.wait_ge(s_cp, _N_SA)
                for r in range(BPC // 128):
                    sync.dma_start(
                        out=out_sa[r * 128 : (r + 1) * 128, :],
                        in_=t_all[:, 0:SA],
                    ).then_inc(s_out, 16)
                sync.wait_ge(s_cp, _N_CHUNKS)
                for r in range(BPC // 128):
                    sync.dma_start(
                        out=out_ta[r * 128 : (r + 1) * 128, :],
                        in_=t_all[:, SA : SA + TA],
                    ).then_inc(s_out, 16)
                sync.wait_ge(s_out, 64)

            @block.tensor
            def _(tensor):
                tensor.wait_ge(s_in, 16)
                for i, (dst_c, src_c, w) in enumerate(chunks):
                    bank = (i % 8) * 512
                    if i >= 8:
                        # PSUM bank reuse: consumer copy must be done
                        tensor.wait_ge(s_cp, i - 7)
                    # out[m, n] = sum_k ones[k, m] * brow[k, src_c + n], K=1
                    tensor.matmul(
                        acc[:, bank : bank + w],
                        brow[0:1, 0:128],
                        brow[0:1, src_c : src_c + w],
                    ).then_inc(s_mm)

            @block.vector
            def _(vector):
                for i, (dst_c, src_c, w) in enumerate(chunks):
                    bank = (i % 8) * 512
                    vector.wait_ge(s_mm, i + 1)
                    vector.tensor_scalar_add(
                        t_all[:, dst_c : dst_c + w], acc[:, bank : bank + w], 1.0
                    ).then_inc(s_cp)

    return nc


def _get_nc():
    if "nc" not in _NC_CACHE:
        _NC_CACHE["nc"] = _build_nc()
    return _NC_CACHE["nc"]


def kernel(**inputs):
    global LAST_EXEC_NS
    sa_bias = np.ascontiguousarray(inputs["sa_bias"], dtype=np.float32)
    ta_bias = np.ascontiguousarray(inputs["ta_bias"], dtype=np.float32)
    bias_cat = np.concatenate(
        [np.ones(128, np.float32), sa_bias.ravel(), ta_bias.ravel()]
    )[None, :]

    nc = _get_nc()
    in_maps = [{"bias_cat": bias_cat} for _ in range(N_CORES)]
    res = run_bass_kernel_spmd(nc, in_maps, list(range(N_CORES)), trace=TRACE)
    LAST_EXEC_NS = res.exec_time_ns

    sa = np.concatenate(
        [r["out_sa"].reshape(BPC, T, J, J, 1) for r in res.results], axis=0
    )
    ta = np.concatenate(
        [r["out_ta"].reshape(BPC, J, T, T, 1) for r in res.results], axis=0
    )
    return sa, ta


# revision 7
# speedup vs baseline: 1.0727x; 1.0727x over previous
"""Trainium2 Bass kernel for nn_DMS_STAttention_48722109006179.

Mathematical note (load-bearing): in the reference, `_attention_layer`
ends with softmax over axis=-1, which is the head dimension of size
H=1.  Softmax over a single-element axis is identically 1.0, so the
entire attention computation (linears, edge gather/scatter, LeakyReLU)
collapses and the outputs are exactly

    sa[b, t, i, j, 0] = 1.0 + sa_bias[t, i, j]
    ta[b, j, s, t, 0] = 1.0 + ta_bias[j, s, t]

independent of `src` and all weights (verified bit-exact against the
jax reference on device).  The kernel is therefore a pure memory-regime
problem: materialize ~58 MB of broadcast output.  Sharding: pure data
parallel over batch B=2048 across 8 cores (256 batch rows per core).

Per-core device program (raw bass — the toolchain here encodes at most
ONE semaphore wait per instruction, which rules out TileContext's
auto-drain; explicit standalone wait_ge instructions are used instead):

  sa path: TensorEngine K=1 ones-matmuls broadcast the sa bias row
    across 128 partitions into PSUM (10 x 484-col chunks); DVE fuses
    the +1.0 into the PSUM->SBUF copy; writes of finished column
    groups stream out behind the PE (two 968-col chunks per write,
    row-block 0 on the SP HWDGE queue, row-block 1 on the ACT queue).
  ta path (small): partition-broadcast DMA read straight from DRAM
    ([1,2200] -> [128,2200], stride-0 source), +1.0 on DVE interleaved
    into idle gaps between sa copies, written early on both queues.

fp32 PE matmuls are exact here (bf16x3 decomposition reconstructs the
fp32 operand exactly when the stationary is 1.0) — verified 0.0 abs
err on hardware.
"""

import numpy as np

import concourse.bass as bass
from concourse import mybir
from concourse.bass_utils import run_bass_kernel_spmd

N_CORES = 8
B = 2048
T = 10
J = 22
SA = T * J * J  # 4840
TA = J * T * T  # 2200
BPC = B // N_CORES  # 256 batch rows per core

_SA_CHUNK = 484  # 10 sa matmul chunks (psum: 484*4 B inside a 2 KB bank)
_N_SA = SA // _SA_CHUNK  # 10
_WUNIT = 2 * _SA_CHUNK  # 968 cols per write unit
_N_WU = SA // _WUNIT  # 5 write units per row block

# test.py hooks (ignored by the grading harness)
TRACE = False
LAST_EXEC_NS = None

_NC_CACHE = {}


def _build_nc():
    nc = bass.Bass()
    f32 = mybir.dt.float32

    # bias_cat layout: [ones(128) | sa_bias(4840) | ta_bias(2200)]
    bias_cat = nc.dram_tensor(
        "bias_cat", [1, 128 + SA + TA], f32, kind="ExternalInput"
    )
    out_sa = nc.dram_tensor("out_sa", [BPC, SA], f32, kind="ExternalOutput")
    out_ta = nc.dram_tensor("out_ta", [BPC, TA], f32, kind="ExternalOutput")

    with (
        nc.semaphore("s_in") as s_in,        # bias row landed in SBUF
        nc.semaphore("s_tain") as s_tain,    # ta broadcast-read landed
        nc.semaphore("s_mm") as s_mm,        # sa matmuls done (1 each)
        nc.semaphore("s_cp") as s_cp,        # sa copies done (1 each)
        nc.semaphore("s_ta") as s_ta,        # ta adds done (1 each)
        nc.semaphore("s_wsp") as s_wsp,      # SP-queue write completions
        nc.semaphore("s_wact") as s_wact,    # ACT-queue write completions
        nc.sbuf_tensor("brow", [1, 128 + SA], f32) as brow,
        nc.sbuf_tensor("t_sa", [128, SA], f32) as t_sa,
        nc.sbuf_tensor("t_ta", [128, TA], f32) as t_ta,
        nc.psum_tensor("acc", [128, 4096], f32) as acc,
    ):
        with nc.Block() as block:

            @block.sync
            def _(sync):
                # ones + sa bias row into SBUF (ta comes via broadcast read)
                sync.dma_start(
                    out=brow[:], in_=bias_cat[0:1, 0 : 128 + SA]
                ).then_inc(s_in, 16)
                # row-block 0 sa writes stream behind the copies
                for k in range(_N_WU):
                    sync.wait_ge(s_cp, 2 * k + 2)
                    sync.dma_start(
                        out=out_sa[0:128, k * _WUNIT : (k + 1) * _WUNIT],
                        in_=t_sa[:, k * _WUNIT : (k + 1) * _WUNIT],
                    ).then_inc(s_wsp, 16)
                # ta row-block 1
                sync.wait_ge(s_ta, 2)
                sync.dma_start(
                    out=out_ta[128:256, :], in_=t_ta[:]
                ).then_inc(s_wsp, 16)
                sync.wait_ge(s_wsp, 16 * (_N_WU + 1))

            @block.scalar
            def _(scalar):
                # partition-broadcast read of ta bias: [1,2200]->[128,2200]
                scalar.dma_start(
                    out=t_ta[:],
                    in_=bias_cat[0:1, 128 + SA : 128 + SA + TA].to_broadcast(
                        (128, TA)
                    ),
                ).then_inc(s_tain, 16)
                # row-block 1 sa writes
                for k in range(_N_WU):
                    scalar.wait_ge(s_cp, 2 * k + 2)
                    scalar.dma_start(
                        out=out_sa[128:256, k * _WUNIT : (k + 1) * _WUNIT],
                        in_=t_sa[:, k * _WUNIT : (k + 1) * _WUNIT],
                    ).then_inc(s_wact, 16)
                # ta row-block 0
                scalar.wait_ge(s_ta, 2)
                scalar.dma_start(
                    out=out_ta[0:128, :], in_=t_ta[:]
                ).then_inc(s_wact, 16)
                scalar.wait_ge(s_wact, 16 * (_N_WU + 1))

            @block.tensor
            def _(tensor):
                tensor.wait_ge(s_in, 16)
                for i in range(_N_SA):
                    bank = (i % 8) * 512
                    if i >= 8:
                        # PSUM bank reuse: consumer copy must be done
                        tensor.wait_ge(s_cp, i - 7)
                    c = i * _SA_CHUNK
                    # out[m,n] = sum_k ones[k,m] * brow[k, 128+c+n], K=1
                    tensor.matmul(
                        acc[:, bank : bank + _SA_CHUNK],
                        brow[0:1, 0:128],
                        brow[0:1, 128 + c : 128 + c + _SA_CHUNK],
                    ).then_inc(s_mm)

            @block.vector
            def _(vector):
                for i in range(_N_SA):
                    bank = (i % 8) * 512
                    c = i * _SA_CHUNK
                    vector.wait_ge(s_mm, i + 1)
                    vector.tensor_scalar_add(
                        t_sa[:, c : c + _SA_CHUNK],
                        acc[:, bank : bank + _SA_CHUNK],
                        1.0,
                    ).then_inc(s_cp)
                    if i == 3:
                        # slot the ta +1 into the gap; read landed long ago
                        vector.wait_ge(s_tain, 16)
                        half = TA // 2  # 1100
                        vector.tensor_scalar_add(
                            t_ta[:, 0:half], t_ta[:, 0:half], 1.0
                        ).then_inc(s_ta)
                        vector.tensor_scalar_add(
                            t_ta[:, half:TA], t_ta[:, half:TA], 1.0
                        ).then_inc(s_ta)

    return nc


def _get_nc():
    if "nc" not in _NC_CACHE:
        _NC_CACHE["nc"] = _build_nc()
    return _NC_CACHE["nc"]


def kernel(**inputs):
    global LAST_EXEC_NS
    sa_bias = np.ascontiguousarray(inputs["sa_bias"], dtype=np.float32)
    ta_bias = np.ascontiguousarray(inputs["ta_bias"], dtype=np.float32)
    bias_cat = np.concatenate(
        [np.ones(128, np.float32), sa_bias.ravel(), ta_bias.ravel()]
    )[None, :]

    nc = _get_nc()
    in_maps = [{"bias_cat": bias_cat} for _ in range(N_CORES)]
    res = run_bass_kernel_spmd(nc, in_maps, list(range(N_CORES)), trace=TRACE)
    LAST_EXEC_NS = res.exec_time_ns

    sa = np.concatenate(
        [r["out_sa"].reshape(BPC, T, J, J, 1) for r in res.results], axis=0
    )
    ta = np.concatenate(
        [r["out_ta"].reshape(BPC, J, T, T, 1) for r in res.results], axis=0
    )
    return sa, ta


# revision 12
# speedup vs baseline: 1.2712x; 1.1850x over previous
"""Trainium2 Bass kernel for nn_DMS_STAttention_48722109006179.

Mathematical note (load-bearing): in the reference, `_attention_layer`
ends with softmax over axis=-1, which is the head dimension of size
H=1.  Softmax over a single-element axis is identically 1.0, so the
entire attention computation (linears, edge gather/scatter, LeakyReLU)
collapses and the outputs are exactly

    sa[b, t, i, j, 0] = 1.0 + sa_bias[t, i, j]
    ta[b, j, s, t, 0] = 1.0 + ta_bias[j, s, t]

independent of `src` and all weights (verified bit-exact against the
jax reference on device).  The kernel is therefore a pure memory-regime
problem: materialize ~58 MB of broadcast output.  Sharding: pure data
parallel over batch B=2048 across 8 cores (256 batch rows per core).

Per-core device program (raw bass — the toolchain here encodes at most
ONE semaphore wait per instruction, which rules out TileContext's
auto-drain; explicit standalone wait_ge instructions are used instead):

  sa path: TensorEngine K=1 ones-matmuls broadcast the sa bias row
    across 128 partitions into PSUM (10 x 484-col chunks); DVE fuses
    the +1.0 into the PSUM->SBUF copy; writes of finished column
    groups stream out behind the PE (two 968-col chunks per write,
    row-block 0 on the SP HWDGE queue, row-block 1 on the ACT queue).
  ta path (small): partition-broadcast DMA read straight from DRAM
    ([1,2200] -> [128,2200], stride-0 source), +1.0 on DVE interleaved
    into idle gaps between sa copies, written early on both queues.

fp32 PE matmuls are exact here (bf16x3 decomposition reconstructs the
fp32 operand exactly when the stationary is 1.0) — verified 0.0 abs
err on hardware.
"""

import numpy as np

import concourse.bass as bass
from concourse import mybir
from concourse.bass_utils import run_bass_kernel_spmd

N_CORES = 8
B = 2048
T = 10
J = 22
SA = T * J * J  # 4840
TA = J * T * T  # 2200
BPC = B // N_CORES  # 256 batch rows per core

_SA_CHUNK = 484  # 10 sa matmul chunks (psum: 484*4 B inside a 2 KB bank)
_N_SA = SA // _SA_CHUNK  # 10
_WUNIT = 2 * _SA_CHUNK  # 968 cols per write unit
_N_WU = SA // _WUNIT  # 5 write units per row block

# test.py hooks (ignored by the grading harness)
TRACE = False
LAST_EXEC_NS = None

_NC_CACHE = {}


def _build_nc():
    nc = bass.Bass()
    f32 = mybir.dt.float32

    # bias_cat layout: [ones(128) | sa_bias(4840) | ta_bias(2200)]
    bias_cat = nc.dram_tensor(
        "bias_cat", [1, 128 + SA + TA], f32, kind="ExternalInput"
    )
    out_sa = nc.dram_tensor("out_sa", [BPC, SA], f32, kind="ExternalOutput")
    out_ta = nc.dram_tensor("out_ta", [BPC, TA], f32, kind="ExternalOutput")

    with (
        nc.semaphore("s_in0") as s_in0,      # bias row landed in SBUF
        nc.semaphore("s_tain") as s_tain,    # ta broadcast-read landed
        nc.semaphore("s_mm") as s_mm,        # sa matmuls done (1 each)
        nc.semaphore("s_cp") as s_cp,        # sa copies done (1 each)
        nc.semaphore("s_ta") as s_ta,        # ta adds done (1 each)
        nc.semaphore("s_wsp") as s_wsp,      # SP-queue write completions
        nc.semaphore("s_wact") as s_wact,    # ACT-queue write completions
        nc.sbuf_tensor("brow", [1, 128 + SA], f32) as brow,
        nc.sbuf_tensor("t_sa", [128, SA], f32) as t_sa,
        nc.sbuf_tensor("t_ta", [128, TA], f32) as t_ta,
        nc.psum_tensor("acc", [128, 4096], f32) as acc,
    ):
        _D0 = 128 + _SA_CHUNK  # ones + sa chunk 0: PE can start on this

        with nc.Block() as block:

            @block.sync
            def _(sync):
                sync.dma_start(
                    out=brow[:], in_=bias_cat[0:1, 0 : 128 + SA]
                ).then_inc(s_in0, 16)
                # row-block 0 sa writes stream behind the copies;
                # ta row-block 1 slots in as soon as the adds are done
                for k in range(_N_WU):
                    sync.wait_ge(s_cp, 2 * k + 2)
                    sync.dma_start(
                        out=out_sa[0:128, k * _WUNIT : (k + 1) * _WUNIT],
                        in_=t_sa[:, k * _WUNIT : (k + 1) * _WUNIT],
                    ).then_inc(s_wsp, 16)
                    if k == 1:
                        sync.wait_ge(s_ta, 2)
                        sync.dma_start(
                            out=out_ta[128:256, :], in_=t_ta[:]
                        ).then_inc(s_wsp, 16)
                sync.wait_ge(s_wsp, 16 * (_N_WU + 1))

            @block.scalar
            def _(scalar):
                # delay the broadcast read until the bias row landed so
                # its 128-descriptor burst can't starve the tiny load
                scalar.wait_ge(s_in0, 16)
                # partition-broadcast read of ta bias: [1,2200]->[128,2200]
                scalar.dma_start(
                    out=t_ta[:],
                    in_=bias_cat[0:1, 128 + SA : 128 + SA + TA].to_broadcast(
                        (128, TA)
                    ),
                ).then_inc(s_tain, 16)
                # row-block 1 sa writes + ta row-block 0
                for k in range(_N_WU):
                    scalar.wait_ge(s_cp, 2 * k + 2)
                    scalar.dma_start(
                        out=out_sa[128:256, k * _WUNIT : (k + 1) * _WUNIT],
                        in_=t_sa[:, k * _WUNIT : (k + 1) * _WUNIT],
                    ).then_inc(s_wact, 16)
                    if k == 1:
                        scalar.wait_ge(s_ta, 2)
                        scalar.dma_start(
                            out=out_ta[0:128, :], in_=t_ta[:]
                        ).then_inc(s_wact, 16)
                scalar.wait_ge(s_wact, 16 * (_N_WU + 1))

            @block.tensor
            def _(tensor):
                tensor.wait_ge(s_in0, 16)
                for i in range(_N_SA):
                    bank = (i % 8) * 512
                    if i >= 8:
                        # PSUM bank reuse: consumer copy must be done
                        tensor.wait_ge(s_cp, i - 7)
                    c = i * _SA_CHUNK
                    # out[m,n] = sum_k ones[k,m] * brow[k, 128+c+n], K=1
                    tensor.matmul(
                        acc[:, bank : bank + _SA_CHUNK],
                        brow[0:1, 0:128],
                        brow[0:1, 128 + c : 128 + c + _SA_CHUNK],
                    ).then_inc(s_mm)

            @block.vector
            def _(vector):
                for i in range(_N_SA):
                    bank = (i % 8) * 512
                    c = i * _SA_CHUNK
                    vector.wait_ge(s_mm, i + 1)
                    vector.tensor_scalar_add(
                        t_sa[:, c : c + _SA_CHUNK],
                        acc[:, bank : bank + _SA_CHUNK],
                        1.0,
                    ).then_inc(s_cp)
                    if i == 1:
                        # slot the ta +1 in early; read lands ~10us
                        vector.wait_ge(s_tain, 16)
                        half = TA // 2  # 1100
                        vector.tensor_scalar_add(
                            t_ta[:, 0:half], t_ta[:, 0:half], 1.0
                        ).then_inc(s_ta)
                        vector.tensor_scalar_add(
                            t_ta[:, half:TA], t_ta[:, half:TA], 1.0
                        ).then_inc(s_ta)

    return nc


def _get_nc():
    if "nc" not in _NC_CACHE:
        _NC_CACHE["nc"] = _build_nc()
    return _NC_CACHE["nc"]


def kernel(**inputs):
    global LAST_EXEC_NS
    sa_bias = np.ascontiguousarray(inputs["sa_bias"], dtype=np.float32)
    ta_bias = np.ascontiguousarray(inputs["ta_bias"], dtype=np.float32)
    bias_cat = np.concatenate(
        [np.ones(128, np.float32), sa_bias.ravel(), ta_bias.ravel()]
    )[None, :]

    nc = _get_nc()
    in_maps = [{"bias_cat": bias_cat} for _ in range(N_CORES)]
    res = run_bass_kernel_spmd(nc, in_maps, list(range(N_CORES)), trace=TRACE)
    LAST_EXEC_NS = res.exec_time_ns

    sa = np.concatenate(
        [r["out_sa"].reshape(BPC, T, J, J, 1) for r in res.results], axis=0
    )
    ta = np.concatenate(
        [r["out_ta"].reshape(BPC, J, T, T, 1) for r in res.results], axis=0
    )
    return sa, ta
